# revision 9
# baseline (speedup 1.0000x reference)
"""DigitCaps dynamic-routing kernel for Trainium2, 8 NeuronCores (SPMD).

Problem:  in_caps [64, 2048, 16] f32, W [2048, 32, 32, 16] f32
          u_hat[b,r,j,o] = sum_i W[r,j,o,i] * in_caps[b,r,i]
          3 routing iterations:
            c = softmax_j(b_ij)                     # b_ij [R, J]
            s[b,j,o] = sum_r c[r,j] u_hat[b,r,j,o]
            v = squash_o(s)
            b_ij += (1/BS) sum_{b,o} u_hat[b,r,j,o] v[b,j,o]
          returns v[..., None]  -> [64, 32, 32, 1]

Strategy (per core, routes sharded 256/core; K = (r,i) = 4096 rows):
  * W shard stays SBUF-resident as Wt[(r,i), (j,o)] = [4096, 1024] (128KB/part).
  * u_hat is never materialized.  Each iteration:
      pass 1: s = (c-scaled Wt).T-contracted with uT -> one K=4096 matmul
              streamed in 32 chunks of K=128 (moving operand = scaled W chunk,
              stationary = uT chunk).  f32r -> full PE rate.
              -> AllReduce of partial s [64, 1024] (the only collective).
      pass 2: G[(r,i),(j,o)] = sum_b u[b,(r,i)] v[b,(j,o)]   (PE, K=64)
              b_upd[r,j] = (1/64) sum_{i,o} Wt . G           (DVE mult +
              o-reduce + i-reduce via a constant block-selector matmul,
              accumulated directly into a persistent PSUM b tensor).
  * c_ij softmax is computed replicated over the 16 i-rows of each route so
    the scale of W chunks is a plain broadcast tensor_tensor multiply.
"""

import numpy as np

import concourse.bacc as bacc
import concourse.mybir as mybir
import concourse.tile as tile
from concourse.bass_utils import run_bass_kernel_spmd

BS, R, J, I, O = 64, 2048, 32, 16, 32
NUM_IT = 3
N_CORES = 8
R_LOC = R // N_CORES            # 256 routes per core
K_LOC = R_LOC * I               # 4096 contraction rows per core
NCHUNK = K_LOC // 128           # 32 chunks of 128 rows (8 routes x 16 i)
JO = J * O                      # 1024
F32 = mybir.dt.float32
F32R = mybir.dt.float32r
AX = mybir.AxisListType
ALU = mybir.AluOpType
ACTF = mybir.ActivationFunctionType

# o-reduction in pass 2: "mm_dup" folds it into the selector matmul via a
# duplicated (step-0) PSUM out AP; "dve" uses an explicit tensor_reduce.
O_REDUCE = "dve"
# chunks whose c-scale multiply runs on gpsimd instead of vector (pass 1)
WC_ON_GPSIMD = lambda c: c % 3 == 0
# chunks whose W.G multiply runs on gpsimd (needs an ACT psum->sbuf copy)
MULT_ON_GPSIMD = lambda c: c % 3 != 2


def _build_nc():
    nc = bacc.Bacc(trn_type="TRN2", target_bir_lowering=False, debug=False,
                   num_devices=N_CORES)
    wt = nc.dram_tensor("wt", [K_LOC, JO], F32R, kind="ExternalInput")
    ut = nc.dram_tensor("ut", [K_LOC, BS], F32R, kind="ExternalInput")
    un = nc.dram_tensor("un", [BS, K_LOC], F32R, kind="ExternalInput")
    sel = nc.dram_tensor("sel", [128, 128], F32, kind="ExternalInput")
    vout = nc.dram_tensor("vout", [BS, JO], F32, kind="ExternalOutput")
    cc_wi = nc.dram_tensor("cc_wi", [1, 128], F32)
    cc_wo = nc.dram_tensor("cc_wo", [1, 128], F32, addr_space="Shared")
    cc_in = [nc.dram_tensor(f"cc_in{i}", [BS, JO], F32) for i in range(NUM_IT)]
    cc_out = [nc.dram_tensor(f"cc_out{i}", [BS, JO], F32, addr_space="Shared")
              for i in range(NUM_IT)]
    rg = [list(range(N_CORES))]

    with tile.TileContext(nc) as tc:
        with (
            tc.tile_pool(name="big", bufs=1) as big,
            tc.tile_pool(name="wc", bufs=3) as wcp,
            tc.tile_pool(name="tmp", bufs=2) as tmpp,
            tc.tile_pool(name="gsb", bufs=2) as gsbp,
            tc.tile_pool(name="small", bufs=1) as small,
            tc.tile_pool(name="spsum", bufs=1, space="PSUM") as spsum,
            tc.tile_pool(name="gpsum", bufs=2, space="PSUM") as gpsum,
            tc.tile_pool(name="bpsum", bufs=1, space="PSUM") as bpsum,
        ):
            # ---- resident tensors ----
            w_sb = big.tile([128, NCHUNK, JO], F32R)       # 128KB/part
            ut_sb = big.tile([128, NCHUNK, BS], F32R)      # 8KB/part
            un_sb = big.tile([BS, K_LOC], F32R)            # 16KB/part
            sel_sb = big.tile([128, 128], F32)            # selector (1/64)
            e_rep = big.tile([128, NCHUNK, J], F32)       # softmax scratch
            c_rep = big.tile([128, NCHUNK, J], F32)       # c_ij replicated
            b_acc = bpsum.tile([128, NCHUNK, J], F32)     # persistent b_ij

            wt_v = wt.ap().rearrange("(c p) f -> c p f", p=128)
            ut_v = ut.ap().rearrange("(c p) f -> c p f", p=128)
            for c in range(NCHUNK):
                nc.sync.dma_start(out=w_sb[:, c, :], in_=wt_v[c])
            for c in range(NCHUNK):
                nc.sync.dma_start(out=ut_sb[:, c, :], in_=ut_v[c])
            nc.sync.dma_start(out=un_sb, in_=un.ap())
            nc.sync.dma_start(out=sel_sb, in_=sel.ap())
            # warm up the collective machinery under the weight load
            nc.gpsimd.collective_compute(
                "AllReduce", ALU.add, replica_groups=rg,
                ins=[cc_wi.ap()], outs=[cc_wo.ap()],
            )

            v_sb = None
            for it in range(NUM_IT):
                # ---------- pass 1: s = sum_{(r,i)} cW . uT ----------
                s_ps = spsum.tile([BS, JO], F32)
                for c in range(NCHUNK):
                    if it == 0:
                        rhs_src = w_sb[:, c, :]
                    else:
                        wc_t = wcp.tile([128, JO], F32R)
                        eng = nc.gpsimd if WC_ON_GPSIMD(c) else nc.vector
                        eng.tensor_tensor(
                            out=wc_t.rearrange("p (j o) -> p j o", o=O),
                            in0=w_sb[:, c, :].bitcast(F32).rearrange("p (j o) -> p j o", o=O),
                            in1=c_rep[:, c, :].unsqueeze(2).broadcast_to(
                                [128, J, O]),
                            op=ALU.mult,
                        )
                        rhs_src = wc_t
                    for h in range(2):
                        nc.tensor.matmul(
                            out=s_ps[:, h * 512:(h + 1) * 512],
                            lhsT=ut_sb[:, c, :],
                            rhs=rhs_src[:, h * 512:(h + 1) * 512],
                            start=(c == 0), stop=(c == NCHUNK - 1),
                        )
                # psum -> sbuf (iter 0 also applies the uniform c = 1/J)
                s_sb = small.tile([BS, JO], F32)
                if it == 0:
                    nc.scalar.mul(s_sb, s_ps, 1.0 / J)
                else:
                    nc.scalar.copy(s_sb, s_ps)

                # ---------- AllReduce over cores ----------
                for q in range(4):
                    nc.sync.dma_start(out=cc_in[it].ap()[:, q * 256:(q + 1) * 256],
                                      in_=s_sb[:, q * 256:(q + 1) * 256])
                nc.gpsimd.collective_compute(
                    "AllReduce", ALU.add, replica_groups=rg,
                    ins=[cc_in[it].ap()], outs=[cc_out[it].ap()],
                )
                s2 = small.tile([BS, J, O], F32)
                s2f = s2.rearrange("p j o -> p (j o)")
                for q in range(4):
                    nc.sync.dma_start(out=s2f[:, q * 256:(q + 1) * 256],
                                      in_=cc_out[it].ap()[:, q * 256:(q + 1) * 256])

                # ---------- squash ----------
                ss = small.tile([BS, J, O], F32, tag="s_sb")
                nc.vector.tensor_tensor(out=ss, in0=s2, in1=s2, op=ALU.mult)
                sq = small.tile([BS, J], F32)
                nc.vector.tensor_reduce(out=sq, in_=ss, axis=AX.X, op=ALU.add)
                rt = small.tile([BS, J], F32)
                nc.scalar.activation(rt, sq, ACTF.Sqrt)       # sqrt(sq)
                op1 = small.tile([BS, J], F32)
                nc.scalar.add(op1, sq, 1.0)                   # 1 + sq
                den = small.tile([BS, J], F32)
                nc.vector.tensor_tensor(out=den, in0=rt, in1=op1, op=ALU.mult)
                rden = small.tile([BS, J], F32)
                nc.vector.reciprocal(rden, den)
                fac = small.tile([BS, J], F32)
                nc.vector.tensor_tensor(out=fac, in0=sq, in1=rden, op=ALU.mult)
                v_sb = small.tile([BS, J, O], F32)
                nc.vector.tensor_tensor(
                    out=v_sb, in0=s2,
                    in1=fac.unsqueeze(2).broadcast_to([BS, J, O]), op=ALU.mult)

                if it == NUM_IT - 1:
                    break

                # ---------- pass 2: b_ij += (1/BS) sum_{i,o} Wt . (uT v) ----
                v_r = small.tile([BS, JO], F32R)
                nc.vector.tensor_copy(v_r, v_sb.rearrange("p j o -> p (j o)"))
                for c in range(NCHUNK):
                    g_ps = gpsum.tile([128, JO], F32)
                    for h in range(2):
                        nc.tensor.matmul(
                            out=g_ps[:, h * 512:(h + 1) * 512],
                            lhsT=un_sb[:, c * 128:(c + 1) * 128],
                            rhs=v_r[:, h * 512:(h + 1) * 512],
                            start=True, stop=True,
                        )
                    w_c = w_sb[:, c, :].bitcast(F32)
                    if MULT_ON_GPSIMD(c):
                        g_sb = gsbp.tile([128, JO], F32)
                        nc.scalar.copy(g_sb, g_ps)
                        g_src, eng = g_sb, nc.gpsimd
                    else:
                        g_src, eng = g_ps, nc.vector
                    tmp = tmpp.tile([128, JO], F32)
                    eng.tensor_tensor(out=tmp, in0=w_c, in1=g_src, op=ALU.mult)
                    tmp3 = tmp.rearrange("p (j o) -> p j o", o=O)
                    if O_REDUCE == "mm_dup":
                        # selector matmul; o-sum via duplicated psum out AP
                        for h in range(2):
                            nc.tensor.matmul(
                                out=b_acc[:, c, h * 16:(h + 1) * 16]
                                    .unsqueeze(2).broadcast_to([128, 16, O]),
                                lhsT=sel_sb,
                                rhs=tmp3[:, h * 16:(h + 1) * 16, :],
                                start=(it == 0 and h == 0 and c % 16 == 0),
                                stop=(it == NUM_IT - 2 and h == 1
                                      and c % 16 == 15),
                                skip_group_check=True,
                            )
                    else:
                        part = tmpp.tile([128, J], F32, tag="part")
                        nc.vector.tensor_reduce(out=part, in_=tmp3, axis=AX.X,
                                                op=ALU.add)
                        nc.tensor.matmul(
                            out=b_acc[:, c, :],
                            lhsT=sel_sb,
                            rhs=part,
                            start=(it == 0 and c % 16 == 0),
                            stop=(it == NUM_IT - 2 and c % 16 == 15),
                            skip_group_check=True,
                        )

                # ---------- softmax over j (replicated rows) ----------
                nc.scalar.activation(e_rep, b_acc, ACTF.Exp)
                esum = small.tile([128, NCHUNK], F32)
                nc.vector.tensor_reduce(out=esum, in_=e_rep, axis=AX.X,
                                        op=ALU.add)
                erec = small.tile([128, NCHUNK], F32)
                nc.vector.reciprocal(erec, esum)
                nc.vector.tensor_tensor(
                    out=c_rep, in0=e_rep,
                    in1=erec.unsqueeze(2).broadcast_to([128, NCHUNK, J]),
                    op=ALU.mult)

            nc.sync.dma_start(out=vout.ap(),
                              in_=v_sb.rearrange("p j o -> p (j o)"))
    nc.finalize()
    return nc


_NC_CACHE = {}
TRACE = False            # test harness sets True for NTFF profiling
TRACE_CORES = None


def _get_nc():
    if "nc" not in _NC_CACHE:
        _NC_CACHE["nc"] = _build_nc()
    return _NC_CACHE["nc"]


def _make_sel():
    sel = np.zeros((128, 128), np.float32)
    for p in range(128):
        m0 = (p // 16) * 16
        sel[p, m0:m0 + 16] = 1.0 / BS
    return sel


def kernel(**inputs):
    in_caps = np.ascontiguousarray(inputs["in_caps"], dtype=np.float32)
    W = np.ascontiguousarray(inputs["W"], dtype=np.float32)
    assert in_caps.shape == (BS, R, I) and W.shape == (R, J, O, I)

    Wt = np.ascontiguousarray(
        W.transpose(0, 3, 1, 2).reshape(R * I, J * O))       # [(r,i), (j,o)]
    uT = np.ascontiguousarray(
        in_caps.transpose(1, 2, 0).reshape(R * I, BS))       # [(r,i), b]
    un = np.ascontiguousarray(in_caps.reshape(BS, R * I))    # [b, (r,i)]
    sel = _make_sel()

    in_maps = []
    for k in range(N_CORES):
        rows = slice(k * K_LOC, (k + 1) * K_LOC)
        in_maps.append({
            "wt": np.ascontiguousarray(Wt[rows]),
            "ut": np.ascontiguousarray(uT[rows]),
            "un": np.ascontiguousarray(un[:, rows]),
            "sel": sel,
        })

    nc = _get_nc()
    res = run_bass_kernel_spmd(nc, in_maps, core_ids=list(range(N_CORES)),
                               trace=TRACE, trace_cores=TRACE_CORES)
    _NC_CACHE["last_result"] = res
    v = np.asarray(res.results[0]["vout"], dtype=np.float32)
    return v.reshape(BS, J, O, 1)


if __name__ == "__main__":
    rng = np.random.default_rng(0)
    ins = {
        "in_caps": rng.standard_normal((BS, R, I), dtype=np.float32),
        "W": rng.standard_normal((R, J, O, I), dtype=np.float32),
    }
    out = kernel(**ins)
    print(out.shape, out.dtype, np.abs(out).mean())


# revision 11
# speedup vs baseline: 1.2125x; 1.2125x over previous
"""DigitCaps dynamic-routing kernel for Trainium2, 8 NeuronCores (SPMD).

Problem:  in_caps [64, 2048, 16] f32, W [2048, 32, 32, 16] f32
          u_hat[b,r,j,o] = sum_i W[r,j,o,i] * in_caps[b,r,i]
          3 routing iterations:
            c = softmax_j(b_ij);  s[b,j,o] = sum_r c[r,j] u_hat[b,r,j,o]
            v = squash_o(s);      b_ij += (1/BS) sum_{b,o} u_hat v
          returns v[..., None]  -> [64, 32, 32, 1]

Strategy (per core, routes sharded 256/core; K = (r,i) = 4096 rows):
  * W shard resident in SBUF as bf16 Wt[(r,i), (j,o)]; u_hat never
    materialized.  Each iteration:
      pass 1: s = (c-scaled Wt) contracted with uT on PE (K=4096, 32 chunks).
              One AllReduce of partial s [64, 1024] per iteration.
      pass 2: G = un.T @ v (PE);  b_upd = (1/64) sum_{i,o} Wt.G via
              DVE mult + o-reduce + i-reduce through a constant selector
              matmul accumulated in a persistent PSUM b_ij.
  * softmax / c-scale replicated over the 16 i-rows per route; the c scale
    is ACT-expanded over o to keep the DVE multiplies in 2x bf16 mode.
  * pass 2 of iteration t emits c chunk-by-chunk so pass 1 of t+1 overlaps.
"""

import numpy as np
import ml_dtypes

import concourse.bacc as bacc
import concourse.mybir as mybir
import concourse.tile as tile
from concourse.bass_utils import run_bass_kernel_spmd

BS, R, J, I, O = 64, 2048, 32, 16, 32
NUM_IT = 3
N_CORES = 8
R_LOC = R // N_CORES            # 256 routes per core
K_LOC = R_LOC * I               # 4096 contraction rows per core
NCHUNK = K_LOC // 128           # 32 chunks (8 routes x 16 i each)
JO = J * O                      # 1024
F32 = mybir.dt.float32
BF16 = mybir.dt.bfloat16
AX = mybir.AxisListType
ALU = mybir.AluOpType
ACTF = mybir.ActivationFunctionType

WC_ON_GPSIMD = lambda c: c % 8 < 5      # 20/32 c-scale chunks on gpsimd
MULT_ON_GPSIMD = lambda c: c % 8 == 7   # 4/32 W.G multiplies on gpsimd


def _build_nc():
    nc = bacc.Bacc(trn_type="TRN2", target_bir_lowering=False, debug=False,
                   num_devices=N_CORES)
    wt = nc.dram_tensor("wt", [K_LOC, JO], BF16, kind="ExternalInput")
    ut = nc.dram_tensor("ut", [K_LOC, BS], BF16, kind="ExternalInput")
    un = nc.dram_tensor("un", [BS, K_LOC], BF16, kind="ExternalInput")
    sel = nc.dram_tensor("sel", [128, 128], F32, kind="ExternalInput")
    vout = nc.dram_tensor("vout", [BS, JO], F32, kind="ExternalOutput")
    cc_wi = nc.dram_tensor("cc_wi", [1, 128], F32)
    cc_wo = nc.dram_tensor("cc_wo", [1, 128], F32, addr_space="Shared")
    cc_in = [nc.dram_tensor(f"cc_in{i}", [BS, JO], F32) for i in range(NUM_IT)]
    cc_out = [nc.dram_tensor(f"cc_out{i}", [BS, JO], F32, addr_space="Shared")
              for i in range(NUM_IT)]
    rg = [list(range(N_CORES))]

    with tile.TileContext(nc) as tc:
        with (
            tc.tile_pool(name="big", bufs=1) as big,
            tc.tile_pool(name="wc", bufs=4) as wcp,
            tc.tile_pool(name="cx", bufs=4) as cxp,
            tc.tile_pool(name="tmp", bufs=3) as tmpp,
            tc.tile_pool(name="gsb", bufs=3) as gsbp,
            tc.tile_pool(name="small", bufs=1) as small,
            tc.tile_pool(name="spsum", bufs=1, space="PSUM") as spsum,
            tc.tile_pool(name="gpsum", bufs=2, space="PSUM") as gpsum,
            tc.tile_pool(name="bpsum", bufs=1, space="PSUM") as bpsum,
        ):
            # ---- resident tensors ----
            w_sb = big.tile([128, NCHUNK, JO], BF16)      # 64KB/part
            ut_sb = big.tile([128, NCHUNK, BS], BF16)
            un_sb = big.tile([BS, K_LOC], BF16)
            sel_sb = big.tile([128, 128], F32)            # selector (1/64)
            e_rep = big.tile([128, NCHUNK, J], F32)       # exp(b) scratch
            c_rep = big.tile([128, NCHUNK, J], F32)       # c_ij replicated
            b_acc = bpsum.tile([128, NCHUNK, J], F32)     # persistent b_ij

            wt_v = wt.ap().rearrange("(c p) f -> c p f", p=128)
            ut_v = ut.ap().rearrange("(c p) f -> c p f", p=128)
            for c in range(NCHUNK):
                nc.sync.dma_start(out=w_sb[:, c, :], in_=wt_v[c])
            for c in range(NCHUNK):
                nc.sync.dma_start(out=ut_sb[:, c, :], in_=ut_v[c])
            nc.sync.dma_start(out=un_sb, in_=un.ap())
            nc.sync.dma_start(out=sel_sb, in_=sel.ap())
            # warm up the collective machinery under the weight load
            nc.gpsimd.collective_compute(
                "AllReduce", ALU.add, replica_groups=rg,
                ins=[cc_wi.ap()], outs=[cc_wo.ap()],
            )

            def emit_pass1(it):
                """c-scale + s-matmul accumulation for iteration `it`."""
                s_ps = spsum.tile([BS, JO], F32)
                for c in range(NCHUNK):
                    if it == 0:
                        rhs_src = w_sb[:, c, :]
                    elif WC_ON_GPSIMD(c):
                        wc_t = wcp.tile([128, JO], BF16)
                        nc.gpsimd.tensor_tensor(
                            out=wc_t.rearrange("p (j o) -> p j o", o=O),
                            in0=w_sb[:, c, :].rearrange("p (j o) -> p j o",
                                                        o=O),
                            in1=c_rep[:, c, :].unsqueeze(2).broadcast_to(
                                [128, J, O]),
                            op=ALU.mult)
                        rhs_src = wc_t
                    else:
                        c_exp = cxp.tile([128, JO], BF16)
                        nc.scalar.copy(
                            c_exp.rearrange("p (j o) -> p j o", o=O),
                            c_rep[:, c, :].unsqueeze(2).broadcast_to(
                                [128, J, O]))
                        wc_t = wcp.tile([128, JO], BF16)
                        nc.vector.tensor_tensor(out=wc_t, in0=w_sb[:, c, :],
                                                in1=c_exp, op=ALU.mult)
                        rhs_src = wc_t
                    for h in range(2):
                        nc.tensor.matmul(
                            out=s_ps[:, h * 512:(h + 1) * 512],
                            lhsT=ut_sb[:, c, :],
                            rhs=rhs_src[:, h * 512:(h + 1) * 512],
                            start=(c == 0), stop=(c == NCHUNK - 1))
                return s_ps

            def emit_ar_squash(it, s_ps):
                """psum->AR->squash; returns v_sb [BS, J, O] f32."""
                s_sb = small.tile([BS, JO], F32, tag="s_sb")
                if it == 0:
                    nc.scalar.mul(s_sb, s_ps, 1.0 / J)
                else:
                    nc.scalar.copy(s_sb, s_ps)
                for q in range(4):
                    nc.sync.dma_start(
                        out=cc_in[it].ap()[:, q * 256:(q + 1) * 256],
                        in_=s_sb[:, q * 256:(q + 1) * 256])
                nc.gpsimd.collective_compute(
                    "AllReduce", ALU.add, replica_groups=rg,
                    ins=[cc_in[it].ap()], outs=[cc_out[it].ap()])
                s2 = small.tile([BS, J, O], F32, tag=f"s2_{it % 2}")
                s2f = s2.rearrange("p j o -> p (j o)")
                for q in range(4):
                    nc.sync.dma_start(
                        out=s2f[:, q * 256:(q + 1) * 256],
                        in_=cc_out[it].ap()[:, q * 256:(q + 1) * 256])
                ss = small.tile([BS, J, O], F32, tag="s_sb")
                nc.vector.tensor_tensor(out=ss, in0=s2, in1=s2, op=ALU.mult)
                sq = small.tile([BS, J], F32)
                nc.vector.tensor_reduce(out=sq, in_=ss, axis=AX.X, op=ALU.add)
                rt = small.tile([BS, J], F32)
                nc.scalar.activation(rt, sq, ACTF.Sqrt)
                op1 = small.tile([BS, J], F32)
                nc.scalar.add(op1, sq, 1.0)
                den = small.tile([BS, J], F32)
                nc.vector.tensor_tensor(out=den, in0=rt, in1=op1, op=ALU.mult)
                rden = small.tile([BS, J], F32)
                nc.vector.reciprocal(rden, den)
                fac = small.tile([BS, J], F32)
                nc.vector.tensor_tensor(out=fac, in0=sq, in1=rden,
                                        op=ALU.mult)
                v_sb = small.tile([BS, J, O], F32, tag=f"v_{it % 2}")
                nc.vector.tensor_tensor(
                    out=v_sb, in0=s2,
                    in1=fac.unsqueeze(2).broadcast_to([BS, J, O]),
                    op=ALU.mult)
                return v_sb

            def emit_pass2(it, v_sb):
                """b_ij update + per-chunk softmax refresh of c_rep."""
                v_r = small.tile([BS, JO], BF16, tag=f"vr{it % 2}")
                nc.vector.tensor_copy(v_r, v_sb.rearrange("p j o -> p (j o)"))
                for c in range(NCHUNK):
                    g_ps = gpsum.tile([128, JO], F32)
                    for h in range(2):
                        nc.tensor.matmul(
                            out=g_ps[:, h * 512:(h + 1) * 512],
                            lhsT=un_sb[:, c * 128:(c + 1) * 128],
                            rhs=v_r[:, h * 512:(h + 1) * 512],
                            start=True, stop=True)
                    g_sb = gsbp.tile([128, JO], BF16)
                    nc.scalar.copy(g_sb, g_ps)
                    tmp = tmpp.tile([128, JO], BF16)
                    eng = nc.gpsimd if MULT_ON_GPSIMD(c) else nc.vector
                    eng.tensor_tensor(out=tmp, in0=w_sb[:, c, :], in1=g_sb,
                                      op=ALU.mult)
                    part = tmpp.tile([128, J], F32, tag="part")
                    nc.vector.tensor_reduce(
                        out=part,
                        in_=tmp.rearrange("p (j o) -> p j o", o=O),
                        axis=AX.X, op=ALU.add)
                    nc.tensor.matmul(
                        out=b_acc[:, c, :], lhsT=sel_sb, rhs=part,
                        start=(it == 0 and c % 16 == 0),
                        stop=(it == NUM_IT - 2 and c % 16 == 15),
                        skip_group_check=True)
                    # per-chunk softmax so iteration it+1 can start early
                    nc.scalar.activation(e_rep[:, c, :], b_acc[:, c, :],
                                         ACTF.Exp)
                    esum = tmpp.tile([128, 1], F32, tag="esum")
                    nc.vector.tensor_reduce(out=esum, in_=e_rep[:, c, :],
                                            axis=AX.X, op=ALU.add)
                    erec = tmpp.tile([128, 1], F32, tag="erec")
                    nc.vector.reciprocal(erec, esum)
                    nc.vector.tensor_tensor(
                        out=c_rep[:, c, :], in0=e_rep[:, c, :],
                        in1=erec.broadcast_to([128, J]), op=ALU.mult)

            v_sb = None
            for it in range(NUM_IT):
                s_ps = emit_pass1(it)
                v_sb = emit_ar_squash(it, s_ps)
                if it < NUM_IT - 1:
                    emit_pass2(it, v_sb)

            nc.sync.dma_start(out=vout.ap(),
                              in_=v_sb.rearrange("p j o -> p (j o)"))
    nc.finalize()
    return nc


_NC_CACHE = {}
TRACE = False
TRACE_CORES = None


def _get_nc():
    if "nc" not in _NC_CACHE:
        _NC_CACHE["nc"] = _build_nc()
    return _NC_CACHE["nc"]


def _make_sel():
    sel = np.zeros((128, 128), np.float32)
    for p in range(128):
        m0 = (p // 16) * 16
        sel[p, m0:m0 + 16] = 1.0 / BS
    return sel


def kernel(**inputs):
    in_caps = np.ascontiguousarray(inputs["in_caps"], dtype=np.float32)
    W = np.ascontiguousarray(inputs["W"], dtype=np.float32)
    assert in_caps.shape == (BS, R, I) and W.shape == (R, J, O, I)

    bf = ml_dtypes.bfloat16
    Wt = np.ascontiguousarray(
        W.transpose(0, 3, 1, 2).reshape(R * I, J * O).astype(bf))
    uT = np.ascontiguousarray(
        in_caps.transpose(1, 2, 0).reshape(R * I, BS).astype(bf))
    un = np.ascontiguousarray(in_caps.reshape(BS, R * I).astype(bf))
    sel = _make_sel()

    in_maps = []
    for k in range(N_CORES):
        rows = slice(k * K_LOC, (k + 1) * K_LOC)
        in_maps.append({
            "wt": np.ascontiguousarray(Wt[rows]),
            "ut": np.ascontiguousarray(uT[rows]),
            "un": np.ascontiguousarray(un[:, rows]),
            "sel": sel,
        })

    nc = _get_nc()
    res = run_bass_kernel_spmd(nc, in_maps, core_ids=list(range(N_CORES)),
                               trace=TRACE, trace_cores=TRACE_CORES)
    _NC_CACHE["last_result"] = res
    v = np.asarray(res.results[0]["vout"], dtype=np.float32)
    return v.reshape(BS, J, O, 1)


if __name__ == "__main__":
    rng = np.random.default_rng(0)
    ins = {
        "in_caps": rng.standard_normal((BS, R, I), dtype=np.float32),
        "W": rng.standard_normal((R, J, O, I), dtype=np.float32),
    }
    out = kernel(**ins)
    print(out.shape, out.dtype, np.abs(out).mean())


# revision 12
# speedup vs baseline: 1.2962x; 1.0690x over previous
"""DigitCaps dynamic-routing kernel for Trainium2, 8 NeuronCores (SPMD).

Problem:  in_caps [64, 2048, 16] f32, W [2048, 32, 32, 16] f32
          u_hat[b,r,j,o] = sum_i W[r,j,o,i] * in_caps[b,r,i]
          3 routing iterations:
            c = softmax_j(b_ij);  s[b,j,o] = sum_r c[r,j] u_hat[b,r,j,o]
            v = squash_o(s);      b_ij += (1/BS) sum_{b,o} u_hat v
          returns v[..., None]  -> [64, 32, 32, 1]

Strategy (per core, routes sharded 256/core; K = (r,i) = 4096 rows):
  * W shard resident in SBUF as bf16 Wt[(r,i), (j,o)]; u_hat never
    materialized.  Each iteration:
      pass 1: s = (c-scaled Wt) contracted with uT on PE (K=4096, 32 chunks).
              One AllReduce of partial s [64, 1024] per iteration.
      pass 2: G = un.T @ v (PE);  b_upd = (1/64) sum_{i,o} Wt.G via
              DVE mult + o-reduce + i-reduce through a constant selector
              matmul accumulated in a persistent PSUM b_ij.
  * softmax / c-scale replicated over the 16 i-rows per route; the c scale
    is ACT-expanded over o to keep the DVE multiplies in 2x bf16 mode.
  * pass 2 of iteration t emits c chunk-by-chunk so pass 1 of t+1 overlaps.
"""

import numpy as np
import ml_dtypes

import concourse.bacc as bacc
import concourse.mybir as mybir
import concourse.tile as tile
from concourse.bass_utils import run_bass_kernel_spmd

BS, R, J, I, O = 64, 2048, 32, 16, 32
NUM_IT = 3
N_CORES = 8
R_LOC = R // N_CORES            # 256 routes per core
K_LOC = R_LOC * I               # 4096 contraction rows per core
NCHUNK = K_LOC // 128           # 32 chunks (8 routes x 16 i each)
JO = J * O                      # 1024
F32 = mybir.dt.float32
BF16 = mybir.dt.bfloat16
FP16 = mybir.dt.float16
AX = mybir.AxisListType
ALU = mybir.AluOpType
ACTF = mybir.ActivationFunctionType

WC_ON_GPSIMD = lambda b: b % 2 == 0     # which 2-chunk wc batches on gpsimd
MULT_ON_GPSIMD = lambda b: b % 8 == 7   # which 2-chunk mult batches on gpsimd
GCOPY_ON_DVE = lambda c: c % 4 == 3     # psum->sbuf g copies moved to vector


def _build_nc():
    nc = bacc.Bacc(trn_type="TRN2", target_bir_lowering=False, debug=False,
                   num_devices=N_CORES)
    wt = nc.dram_tensor("wt", [K_LOC, JO], BF16, kind="ExternalInput")
    ut = nc.dram_tensor("ut", [K_LOC, BS], BF16, kind="ExternalInput")
    un = nc.dram_tensor("un", [BS, K_LOC], BF16, kind="ExternalInput")
    sel = nc.dram_tensor("sel", [128, 128], F32, kind="ExternalInput")
    vout = nc.dram_tensor("vout", [BS, JO], F32, kind="ExternalOutput")
    cc_wi = nc.dram_tensor("cc_wi", [1, 128], F32)
    cc_wo = nc.dram_tensor("cc_wo", [1, 128], F32, addr_space="Shared")
    cc_in = [nc.dram_tensor(f"cc_in{i}", [BS, JO], F32) for i in range(NUM_IT)]
    cc_out = [nc.dram_tensor(f"cc_out{i}", [BS, JO], F32, addr_space="Shared")
              for i in range(NUM_IT)]
    rg = [list(range(N_CORES))]

    with tile.TileContext(nc) as tc:
        with (
            tc.tile_pool(name="big", bufs=1) as big,
            tc.tile_pool(name="wc", bufs=4) as wcp,
            tc.tile_pool(name="cx", bufs=4) as cxp,
            tc.tile_pool(name="tmp", bufs=3) as tmpp,
            tc.tile_pool(name="gsb", bufs=3) as gsbp,
            tc.tile_pool(name="small", bufs=1) as small,
            tc.tile_pool(name="spsum", bufs=1, space="PSUM") as spsum,
            tc.tile_pool(name="gpsum", bufs=2, space="PSUM") as gpsum,
            tc.tile_pool(name="bpsum", bufs=1, space="PSUM") as bpsum,
        ):
            # ---- resident tensors ----
            w_sb = big.tile([128, NCHUNK, JO], BF16)      # 64KB/part
            ut_sb = big.tile([128, NCHUNK, BS], BF16)
            un_sb = big.tile([BS, K_LOC], BF16)
            sel_sb = big.tile([128, 128], F32)            # selector (1/64)
            e_rep = big.tile([128, NCHUNK, J], F32)       # exp(b) scratch
            c_rep = big.tile([128, NCHUNK, J], F32)       # c_ij replicated
            b_acc = bpsum.tile([128, NCHUNK, J], F32)     # persistent b_ij

            wt_v = wt.ap().rearrange("(c p) f -> c p f", p=128)
            ut_v = ut.ap().rearrange("(c p) f -> c p f", p=128)
            for c in range(NCHUNK):
                nc.sync.dma_start(out=w_sb[:, c, :], in_=wt_v[c])
            for c in range(NCHUNK):
                nc.sync.dma_start(out=ut_sb[:, c, :], in_=ut_v[c])
            nc.sync.dma_start(out=un_sb, in_=un.ap())
            nc.sync.dma_start(out=sel_sb, in_=sel.ap())
            # warm up the collective machinery under the weight load
            nc.gpsimd.collective_compute(
                "AllReduce", ALU.add, replica_groups=rg,
                ins=[cc_wi.ap()], outs=[cc_wo.ap()],
            )

            def emit_pass1(it):
                """c-scale + s-matmul accumulation for iteration `it`."""
                s_ps = spsum.tile([BS, JO], F32)
                for b in range(NCHUNK // 2):
                    c0 = 2 * b
                    if it == 0:
                        rhs_src = w_sb[:, c0:c0 + 2, :]
                    elif WC_ON_GPSIMD(b):
                        wc_t = wcp.tile([128, 2, JO], BF16)
                        nc.gpsimd.tensor_tensor(
                            out=wc_t.rearrange("p c (j o) -> p c j o", o=O),
                            in0=w_sb[:, c0:c0 + 2, :].rearrange(
                                "p c (j o) -> p c j o", o=O),
                            in1=c_rep[:, c0:c0 + 2, :].unsqueeze(3)
                                .broadcast_to([128, 2, J, O]),
                            op=ALU.mult)
                        rhs_src = wc_t
                    else:
                        c_exp = cxp.tile([128, 2, JO], BF16)
                        nc.scalar.copy(
                            c_exp.rearrange("p c (j o) -> p c j o", o=O),
                            c_rep[:, c0:c0 + 2, :].unsqueeze(3)
                                .broadcast_to([128, 2, J, O]))
                        wc_t = wcp.tile([128, 2, JO], BF16)
                        nc.vector.tensor_tensor(
                            out=wc_t, in0=w_sb[:, c0:c0 + 2, :], in1=c_exp,
                            op=ALU.mult)
                        rhs_src = wc_t
                    for ci in range(2):
                        for h in range(2):
                            nc.tensor.matmul(
                                out=s_ps[:, h * 512:(h + 1) * 512],
                                lhsT=ut_sb[:, c0 + ci, :],
                                rhs=rhs_src[:, ci, h * 512:(h + 1) * 512],
                                start=(c0 + ci == 0),
                                stop=(c0 + ci == NCHUNK - 1))
                return s_ps

            def emit_ar_squash(it, s_ps):
                """psum->AR->squash; returns v_sb [BS, J, O] f32."""
                s_sb = small.tile([BS, JO], F32, tag="s_sb")
                if it == 0:
                    nc.scalar.mul(s_sb, s_ps, 1.0 / J)
                else:
                    nc.scalar.copy(s_sb, s_ps)
                for q in range(4):
                    nc.sync.dma_start(
                        out=cc_in[it].ap()[:, q * 256:(q + 1) * 256],
                        in_=s_sb[:, q * 256:(q + 1) * 256])
                nc.gpsimd.collective_compute(
                    "AllReduce", ALU.add, replica_groups=rg,
                    ins=[cc_in[it].ap()], outs=[cc_out[it].ap()])
                s2 = small.tile([BS, J, O], F32, tag=f"s2_{it % 2}")
                s2f = s2.rearrange("p j o -> p (j o)")
                for q in range(4):
                    nc.sync.dma_start(
                        out=s2f[:, q * 256:(q + 1) * 256],
                        in_=cc_out[it].ap()[:, q * 256:(q + 1) * 256])
                ss = small.tile([BS, J, O], F32, tag="s_sb")
                nc.scalar.square(ss, s2)
                sq = small.tile([BS, J], F32)
                nc.vector.tensor_reduce(out=sq, in_=ss, axis=AX.X, op=ALU.add)
                rt = small.tile([BS, J], F32)
                nc.scalar.activation(rt, sq, ACTF.Sqrt)
                op1 = small.tile([BS, J], F32)
                nc.scalar.add(op1, sq, 1.0)
                den = small.tile([BS, J], F32)
                nc.vector.tensor_tensor(out=den, in0=rt, in1=op1, op=ALU.mult)
                rden = small.tile([BS, J], F32)
                nc.vector.reciprocal(rden, den)
                fac = small.tile([BS, J], F32)
                nc.vector.tensor_tensor(out=fac, in0=sq, in1=rden,
                                        op=ALU.mult)
                v_sb = small.tile([BS, J, O], F32, tag=f"v_{it % 2}")
                nc.vector.tensor_tensor(
                    out=v_sb, in0=s2,
                    in1=fac.unsqueeze(2).broadcast_to([BS, J, O]),
                    op=ALU.mult)
                return v_sb

            def emit_pass2(it, v_sb):
                """b_ij update + grouped softmax refresh of c_rep."""
                v_r = small.tile([BS, JO], BF16, tag=f"vr{it % 2}")
                nc.scalar.copy(v_r, v_sb.rearrange("p j o -> p (j o)"))
                for b in range(NCHUNK // 2):
                    c0 = 2 * b
                    g_sb = gsbp.tile([128, 2, JO], BF16)
                    for ci in range(2):
                        g_ps = gpsum.tile([128, JO], F32)
                        for h in range(2):
                            nc.tensor.matmul(
                                out=g_ps[:, h * 512:(h + 1) * 512],
                                lhsT=un_sb[:, (c0 + ci) * 128:
                                           (c0 + ci + 1) * 128],
                                rhs=v_r[:, h * 512:(h + 1) * 512],
                                start=True, stop=True)
                        ceng = nc.vector if GCOPY_ON_DVE(c0 + ci) else nc.scalar
                        if ceng is nc.vector:
                            ceng.tensor_copy(g_sb[:, ci, :], g_ps)
                        else:
                            ceng.copy(g_sb[:, ci, :], g_ps)
                    tmp = tmpp.tile([128, 2, JO], FP16)
                    eng = nc.gpsimd if MULT_ON_GPSIMD(b) else nc.vector
                    eng.tensor_tensor(out=tmp, in0=w_sb[:, c0:c0 + 2, :],
                                      in1=g_sb, op=ALU.mult)
                    # o-reduction: 5-stage pairwise tree (fp16, 2x mode)
                    t4 = tmp.rearrange("p c (j o) -> p c j o", o=O)
                    w = O
                    while w > 2:
                        h = w // 2
                        nc.vector.tensor_tensor(
                            out=t4[:, :, :, 0:h], in0=t4[:, :, :, 0:h],
                            in1=t4[:, :, :, h:w], op=ALU.add)
                        w = h
                    part = tmpp.tile([128, 2, J], F32, tag="part")
                    nc.vector.tensor_tensor(
                        out=part, in0=t4[:, :, :, 0], in1=t4[:, :, :, 1],
                        op=ALU.add)
                    nc.tensor.matmul(
                        out=b_acc[:, c0:c0 + 2, :], lhsT=sel_sb, rhs=part,
                        start=(it == 0 and c0 % 16 == 0),
                        stop=(it == NUM_IT - 2 and c0 % 16 == 14),
                        skip_group_check=True)
                    # grouped softmax refresh every 4 batches (8 chunks)
                    if b % 4 == 3:
                        gc0 = c0 - 6
                        nc.scalar.activation(e_rep[:, gc0:gc0 + 8, :],
                                             b_acc[:, gc0:gc0 + 8, :],
                                             ACTF.Exp)
                        esum = tmpp.tile([128, 8], F32, tag="esum")
                        nc.vector.tensor_reduce(
                            out=esum, in_=e_rep[:, gc0:gc0 + 8, :],
                            axis=AX.X, op=ALU.add)
                        erec = tmpp.tile([128, 8], F32, tag="erec")
                        nc.vector.reciprocal(erec, esum)
                        for cc in range(8):
                            nc.scalar.mul(c_rep[:, gc0 + cc, :],
                                          e_rep[:, gc0 + cc, :],
                                          erec[:, cc:cc + 1])

            v_sb = None
            for it in range(NUM_IT):
                s_ps = emit_pass1(it)
                v_sb = emit_ar_squash(it, s_ps)
                if it < NUM_IT - 1:
                    emit_pass2(it, v_sb)

            v_flat_out = v_sb.rearrange("p j o -> p (j o)")
            for q in range(4):
                nc.sync.dma_start(out=vout.ap()[:, q * 256:(q + 1) * 256],
                                  in_=v_flat_out[:, q * 256:(q + 1) * 256])
    nc.finalize()
    return nc


_NC_CACHE = {}
TRACE = False
TRACE_CORES = None


def _get_nc():
    if "nc" not in _NC_CACHE:
        _NC_CACHE["nc"] = _build_nc()
    return _NC_CACHE["nc"]


def _make_sel():
    sel = np.zeros((128, 128), np.float32)
    for p in range(128):
        m0 = (p // 16) * 16
        sel[p, m0:m0 + 16] = 1.0 / BS
    return sel


def kernel(**inputs):
    in_caps = np.ascontiguousarray(inputs["in_caps"], dtype=np.float32)
    W = np.ascontiguousarray(inputs["W"], dtype=np.float32)
    assert in_caps.shape == (BS, R, I) and W.shape == (R, J, O, I)

    bf = ml_dtypes.bfloat16
    Wt = np.ascontiguousarray(
        W.transpose(0, 3, 1, 2).reshape(R * I, J * O).astype(bf))
    uT = np.ascontiguousarray(
        in_caps.transpose(1, 2, 0).reshape(R * I, BS).astype(bf))
    un = np.ascontiguousarray(in_caps.reshape(BS, R * I).astype(bf))
    sel = _make_sel()

    in_maps = []
    for k in range(N_CORES):
        rows = slice(k * K_LOC, (k + 1) * K_LOC)
        in_maps.append({
            "wt": np.ascontiguousarray(Wt[rows]),
            "ut": np.ascontiguousarray(uT[rows]),
            "un": np.ascontiguousarray(un[:, rows]),
            "sel": sel,
        })

    nc = _get_nc()
    res = run_bass_kernel_spmd(nc, in_maps, core_ids=list(range(N_CORES)),
                               trace=TRACE, trace_cores=TRACE_CORES)
    _NC_CACHE["last_result"] = res
    v = np.asarray(res.results[0]["vout"], dtype=np.float32)
    return v.reshape(BS, J, O, 1)


if __name__ == "__main__":
    rng = np.random.default_rng(0)
    ins = {
        "in_caps": rng.standard_normal((BS, R, I), dtype=np.float32),
        "W": rng.standard_normal((R, J, O, I), dtype=np.float32),
    }
    out = kernel(**ins)
    print(out.shape, out.dtype, np.abs(out).mean())


# revision 27
# speedup vs baseline: 1.3767x; 1.0621x over previous
"""DigitCaps dynamic-routing kernel for Trainium2, 8 NeuronCores (SPMD).

Problem:  in_caps [64, 2048, 16] f32, W [2048, 32, 32, 16] f32
          u_hat[b,r,j,o] = sum_i W[r,j,o,i] * in_caps[b,r,i]
          3 routing iterations:
            c = softmax_j(b_ij);  s[b,j,o] = sum_r c[r,j] u_hat[b,r,j,o]
            v = squash_o(s);      b_ij += (1/BS) sum_{b,o} u_hat v
          returns v[..., None]  -> [64, 32, 32, 1]

Strategy (per core, routes sharded 256/core; K = (r,i) = 4096 rows):
  * W shard resident in SBUF as bf16 Wt[(r,i), (j,o)]; u_hat never
    materialized.  Each iteration:
      pass 1: s = (c-scaled Wt) contracted with uT on PE (K=4096, 32 chunks).
              One AllReduce of partial s [64, 1024] per iteration.
      pass 2: G = un.T @ v (PE);  b_upd = (1/64) sum_{i,o} Wt.G via
              DVE mult + o-reduce + i-reduce through a constant selector
              matmul accumulated in a persistent PSUM b_ij.
  * softmax / c-scale replicated over the 16 i-rows per route; the c scale
    is ACT-expanded over o to keep the DVE multiplies in 2x bf16 mode.
  * pass 2 of iteration t emits c chunk-by-chunk so pass 1 of t+1 overlaps.
"""

import numpy as np
import ml_dtypes

import concourse.bacc as bacc
import concourse.mybir as mybir
import concourse.tile as tile
from concourse.bass_utils import run_bass_kernel_spmd

BS, R, J, I, O = 64, 2048, 32, 16, 32
NUM_IT = 3
N_CORES = 8
R_LOC = R // N_CORES            # 256 routes per core
K_LOC = R_LOC * I               # 4096 contraction rows per core
NCHUNK = K_LOC // 128           # 32 chunks (8 routes x 16 i each)
JO = J * O                      # 1024
F32 = mybir.dt.float32
BF16 = mybir.dt.bfloat16
FP16 = mybir.dt.float16
AX = mybir.AxisListType
ALU = mybir.AluOpType
ACTF = mybir.ActivationFunctionType

WC_ON_GPSIMD = lambda b: b < 5          # 4-chunk wc batches: 5 gps-direct, 3 ACT-expand+DVE
TREE_ON_GPSIMD = lambda b: False        # o-reduce trees stay on vector
MULT_ON_GPSIMD = lambda b: False        # all W.G multiplies on vector


def _build_nc():
    nc = bacc.Bacc(trn_type="TRN2", target_bir_lowering=False, debug=False,
                   num_devices=N_CORES)
    wt = nc.dram_tensor("wt", [K_LOC, JO], FP16, kind="ExternalInput")
    ut = nc.dram_tensor("ut", [K_LOC, BS], FP16, kind="ExternalInput")
    un = nc.dram_tensor("un", [BS, K_LOC], FP16, kind="ExternalInput")
    sel = nc.dram_tensor("sel", [128, 128], FP16, kind="ExternalInput")
    vout = nc.dram_tensor("vout", [BS, JO], F32, kind="ExternalOutput")
    cc_wi = nc.dram_tensor("cc_wi", [1, 128], F32)
    cc_wo = nc.dram_tensor("cc_wo", [1, 128], F32, addr_space="Shared")
    cc_in = [nc.dram_tensor(f"cc_in{i}", [BS, JO], FP16) for i in range(NUM_IT)]
    cc_out = [nc.dram_tensor(f"cc_out{i}", [BS, JO], FP16, addr_space="Shared")
              for i in range(NUM_IT)]
    rg = [list(range(N_CORES))]

    with tile.TileContext(nc) as tc:
        with (
            tc.tile_pool(name="big", bufs=1) as big,
            tc.tile_pool(name="wc", bufs=2) as wcp,
            tc.tile_pool(name="cx", bufs=4) as cxp,
            tc.tile_pool(name="tmp", bufs=2) as tmpp,
            tc.tile_pool(name="gsb", bufs=2) as gsbp,
            tc.tile_pool(name="small", bufs=1) as small,
            tc.tile_pool(name="ps", bufs=3, space="PSUM") as psp,
            tc.tile_pool(name="bpsum", bufs=1, space="PSUM") as bpsum,
        ):
            # ---- resident tensors ----
            w_sb = big.tile([128, NCHUNK, JO], FP16)      # 64KB/part
            ut_sb = big.tile([128, NCHUNK, BS], FP16)
            un_sb = big.tile([BS, K_LOC], FP16)
            sel_sb = big.tile([128, 128], FP16)            # selector (1/64)
            e_rep = big.tile([128, NCHUNK, J], F32)       # exp(b) scratch
            c_rep = big.tile([128, NCHUNK, J], F32)       # c_ij replicated
            b_acc = bpsum.tile([128, NCHUNK, J], F32)     # persistent b_ij

            wt_v = wt.ap().rearrange("(c p) f -> c p f", p=128)
            ut_v = ut.ap().rearrange("(c p) f -> c p f", p=128)
            _dengs = [nc.sync, nc.scalar, nc.gpsimd]
            for c in range(NCHUNK):
                _dengs[c % 3].dma_start(out=w_sb[:, c, :], in_=wt_v[c])
            for c in range(NCHUNK):
                _dengs[c % 3].dma_start(out=ut_sb[:, c, :], in_=ut_v[c])
            nc.sync.dma_start(out=un_sb, in_=un.ap())
            nc.sync.dma_start(out=sel_sb, in_=sel.ap())
            # warm up the collective machinery under the weight load
            nc.gpsimd.collective_compute(
                "AllReduce", ALU.add, replica_groups=rg,
                ins=[cc_wi.ap()], outs=[cc_wo.ap()],
            )

            def emit_pass1(it):
                """c-scale + s-matmul accumulation for iteration `it`."""
                s_full = psp.tile([128, JO], F32, tag="ps")
                s_ps = s_full[:BS, :]
                for b in range(NCHUNK // 4):
                    c0 = 4 * b
                    if it == 0:
                        rhs_src = w_sb[:, c0:c0 + 4, :]
                    elif WC_ON_GPSIMD(b):
                        wc_t = wcp.tile([128, 4, JO], FP16)
                        nc.gpsimd.tensor_tensor(
                            out=wc_t.rearrange("p c (j o) -> p c j o", o=O),
                            in0=w_sb[:, c0:c0 + 4, :].rearrange(
                                "p c (j o) -> p c j o", o=O),
                            in1=c_rep[:, c0:c0 + 4, :].unsqueeze(3)
                                .broadcast_to([128, 4, J, O]),
                            op=ALU.mult)
                        rhs_src = wc_t
                    else:
                        c_exp = cxp.tile([128, 4, JO], FP16)
                        nc.scalar.copy(
                            c_exp.rearrange("p c (j o) -> p c j o", o=O),
                            c_rep[:, c0:c0 + 4, :].unsqueeze(3)
                                .broadcast_to([128, 4, J, O]))
                        wc_t = wcp.tile([128, 4, JO], FP16)
                        nc.vector.tensor_tensor(
                            out=wc_t, in0=w_sb[:, c0:c0 + 4, :], in1=c_exp,
                            op=ALU.mult)
                        rhs_src = wc_t
                    for ci in range(4):
                        for h in range(2):
                            nc.tensor.matmul(
                                out=s_ps[:, h * 512:(h + 1) * 512],
                                lhsT=ut_sb[:, c0 + ci, :],
                                rhs=rhs_src[:, ci, h * 512:(h + 1) * 512],
                                start=(c0 + ci == 0),
                                stop=(c0 + ci == NCHUNK - 1))
                return s_ps

            def emit_ar_squash(it, s_ps):
                """psum->AR->squash; returns v_sb [BS, J, O] f32."""
                s_sb = small.tile([BS, JO], FP16, tag="s_sb")
                if it == 0:
                    nc.scalar.mul(s_sb, s_ps, 1.0 / J)
                else:
                    nc.scalar.copy(s_sb, s_ps)
                for q in range(4):
                    deng = nc.sync if q % 2 == 0 else nc.scalar
                    deng.dma_start(
                        out=cc_in[it].ap()[:, q * 256:(q + 1) * 256],
                        in_=s_sb[:, q * 256:(q + 1) * 256])
                nc.gpsimd.collective_compute(
                    "AllReduce", ALU.add, replica_groups=rg,
                    ins=[cc_in[it].ap()], outs=[cc_out[it].ap()])
                s2 = small.tile([BS, J, O], FP16, tag=f"s2_{it % 2}")
                s2f = s2.rearrange("p j o -> p (j o)")
                for q in range(4):
                    deng = nc.sync if q % 2 == 0 else nc.scalar
                    deng.dma_start(
                        out=s2f[:, q * 256:(q + 1) * 256],
                        in_=cc_out[it].ap()[:, q * 256:(q + 1) * 256])
                ss = small.tile([BS, J, O], F32, tag="s_sb")
                nc.scalar.square(ss, s2)
                sq = small.tile([BS, J], F32)
                nc.vector.tensor_reduce(out=sq, in_=ss, axis=AX.X, op=ALU.add)
                rt = small.tile([BS, J], F32)
                nc.scalar.activation(rt, sq, ACTF.Sqrt)
                op1 = small.tile([BS, J], F32)
                nc.vector.tensor_scalar_add(op1, sq, 1.0)
                rden = small.tile([BS, J], F32)
                nc.vector.reciprocal(rden, op1)
                fac = small.tile([BS, J], F32)
                nc.vector.tensor_tensor(out=fac, in0=rt, in1=rden,
                                        op=ALU.mult)
                v_sb = small.tile([BS, J, O], F32, tag=f"v_{it % 2}")
                nc.vector.tensor_tensor(
                    out=v_sb, in0=s2,
                    in1=fac.unsqueeze(2).broadcast_to([BS, J, O]),
                    op=ALU.mult)
                return v_sb

            def emit_pass2(it, v_sb):
                """b_ij update + per-batch softmax refresh of c_rep."""
                v_r = small.tile([BS, JO], FP16, tag=f"vr{it % 2}")
                nc.scalar.copy(v_r, v_sb.rearrange("p j o -> p (j o)"))
                NB = NCHUNK // 4
                for b in range(NB):
                    c0 = 4 * b
                    g_sb = gsbp.tile([128, 4, JO], FP16)
                    for ci in range(4):
                        g_ps = psp.tile([128, JO], F32, tag="ps")
                        for h in range(2):
                            nc.tensor.matmul(
                                out=g_ps[:, h * 512:(h + 1) * 512],
                                lhsT=un_sb[:, (c0 + ci) * 128:
                                           (c0 + ci + 1) * 128],
                                rhs=v_r[:, h * 512:(h + 1) * 512],
                                start=True, stop=True)
                        nc.scalar.copy(g_sb[:, ci, :], g_ps)
                    tmp = tmpp.tile([128, 4, JO], FP16)
                    meng = nc.gpsimd if MULT_ON_GPSIMD(b) else nc.vector
                    meng.tensor_tensor(out=tmp, in0=w_sb[:, c0:c0 + 4, :],
                                       in1=g_sb, op=ALU.mult)
                    # o-reduction: two pairwise stages then selector matmuls
                    teng = nc.gpsimd if TREE_ON_GPSIMD(b) else nc.vector
                    ta = tmpp.tile([128, 128, 16], FP16, tag="ta")
                    t0 = tmp.rearrange("p c (j o) -> p (c j) o", o=O)
                    teng.tensor_tensor(out=ta, in0=t0[:, :, 0:16],
                                       in1=t0[:, :, 16:32], op=ALU.add)
                    tb = tmpp.tile([128, 128, 8], FP16, tag="tb")
                    teng.tensor_tensor(out=tb, in0=ta[:, :, 0:8],
                                       in1=ta[:, :, 8:16], op=ALU.add)
                    tcq = tmpp.tile([128, 128, 4], FP16, tag="tc")
                    teng.tensor_tensor(out=tcq, in0=tb[:, :, 0:4],
                                       in1=tb[:, :, 4:8], op=ALU.add)
                    td = tmpp.tile([128, 128, 2], FP16, tag="td")
                    teng.tensor_tensor(out=td, in0=tcq[:, :, 0:2],
                                       in1=tcq[:, :, 2:4], op=ALU.add)
                    for oo in range(2):
                        nc.tensor.matmul(
                            out=b_acc[:, c0:c0 + 4, :], lhsT=sel_sb,
                            rhs=td[:, :, oo],
                            start=(it == 0 and c0 % 16 == 0 and oo == 0),
                            stop=(it == NUM_IT - 2 and c0 % 16 == 12
                                  and oo == 1),
                            skip_group_check=True)
                    # softmax refresh for this batch (4 chunks)
                    nc.scalar.activation(e_rep[:, c0:c0 + 4, :],
                                         b_acc[:, c0:c0 + 4, :], ACTF.Exp)
                    esum = tmpp.tile([128, 4], F32, tag="esum")
                    nc.vector.tensor_reduce(
                        out=esum, in_=e_rep[:, c0:c0 + 4, :],
                        axis=AX.X, op=ALU.add)
                    erec = tmpp.tile([128, 4], F32, tag="erec")
                    nc.vector.reciprocal(erec, esum)
                    for cc in range(4):
                        nc.scalar.mul(c_rep[:, c0 + cc, :],
                                      e_rep[:, c0 + cc, :],
                                      erec[:, cc:cc + 1])

            v_sb = None
            for it in range(NUM_IT):
                s_ps = emit_pass1(it)
                v_sb = emit_ar_squash(it, s_ps)
                if it < NUM_IT - 1:
                    emit_pass2(it, v_sb)

            v_flat_out = v_sb.rearrange("p j o -> p (j o)")
            for q in range(4):
                nc.sync.dma_start(out=vout.ap()[:, q * 256:(q + 1) * 256],
                                  in_=v_flat_out[:, q * 256:(q + 1) * 256])
    nc.finalize()
    return nc


_NC_CACHE = {}
TRACE = False
TRACE_CORES = None


def _get_nc():
    if "nc" not in _NC_CACHE:
        _NC_CACHE["nc"] = _build_nc()
    return _NC_CACHE["nc"]


def _make_sel():
    sel = np.zeros((128, 128), np.float32)
    for p in range(128):
        m0 = (p // 16) * 16
        sel[p, m0:m0 + 16] = 1.0 / BS
    return sel


def kernel(**inputs):
    in_caps = np.ascontiguousarray(inputs["in_caps"], dtype=np.float32)
    W = np.ascontiguousarray(inputs["W"], dtype=np.float32)
    assert in_caps.shape == (BS, R, I) and W.shape == (R, J, O, I)

    bf = np.float16
    Wt = np.ascontiguousarray(
        W.transpose(0, 3, 1, 2).reshape(R * I, J * O).astype(bf))
    uT = np.ascontiguousarray(
        in_caps.transpose(1, 2, 0).reshape(R * I, BS).astype(bf))
    un = np.ascontiguousarray(in_caps.reshape(BS, R * I).astype(bf))
    sel = _make_sel().astype(np.float16)

    in_maps = []
    for k in range(N_CORES):
        rows = slice(k * K_LOC, (k + 1) * K_LOC)
        in_maps.append({
            "wt": np.ascontiguousarray(Wt[rows]),
            "ut": np.ascontiguousarray(uT[rows]),
            "un": np.ascontiguousarray(un[:, rows]),
            "sel": sel,
        })

    nc = _get_nc()
    res = run_bass_kernel_spmd(nc, in_maps, core_ids=list(range(N_CORES)),
                               trace=TRACE, trace_cores=TRACE_CORES)
    _NC_CACHE["last_result"] = res
    v = np.asarray(res.results[0]["vout"], dtype=np.float32)
    return v.reshape(BS, J, O, 1)


if __name__ == "__main__":
    rng = np.random.default_rng(0)
    ins = {
        "in_caps": rng.standard_normal((BS, R, I), dtype=np.float32),
        "W": rng.standard_normal((R, J, O, I), dtype=np.float32),
    }
    out = kernel(**ins)
    print(out.shape, out.dtype, np.abs(out).mean())


# revision 29
# speedup vs baseline: 1.4640x; 1.0634x over previous
"""DigitCaps dynamic-routing kernel for Trainium2, 8 NeuronCores (SPMD).

Problem:  in_caps [64, 2048, 16] f32, W [2048, 32, 32, 16] f32
          u_hat[b,r,j,o] = sum_i W[r,j,o,i] * in_caps[b,r,i]
          3 routing iterations:
            c = softmax_j(b_ij);  s[b,j,o] = sum_r c[r,j] u_hat[b,r,j,o]
            v = squash_o(s);      b_ij += (1/BS) sum_{b,o} u_hat v
          returns v[..., None]  -> [64, 32, 32, 1]

Strategy (per core, routes sharded 256/core; K = (r,i) = 4096 rows):
  * W shard resident in SBUF as bf16 Wt[(r,i), (j,o)]; u_hat never
    materialized.  Each iteration:
      pass 1: s = (c-scaled Wt) contracted with uT on PE (K=4096, 32 chunks).
              One AllReduce of partial s [64, 1024] per iteration.
      pass 2: G = un.T @ v (PE);  b_upd = (1/64) sum_{i,o} Wt.G via
              DVE mult + o-reduce + i-reduce through a constant selector
              matmul accumulated in a persistent PSUM b_ij.
  * softmax / c-scale replicated over the 16 i-rows per route; the c scale
    is ACT-expanded over o to keep the DVE multiplies in 2x bf16 mode.
  * pass 2 of iteration t emits c chunk-by-chunk so pass 1 of t+1 overlaps.
"""

import numpy as np
import ml_dtypes

import concourse.bacc as bacc
import concourse.mybir as mybir
import concourse.tile as tile
from concourse.bass_utils import run_bass_kernel_spmd

BS, R, J, I, O = 64, 2048, 32, 16, 32
NUM_IT = 3
N_CORES = 8
R_LOC = R // N_CORES            # 256 routes per core
K_LOC = R_LOC * I               # 4096 contraction rows per core
NCHUNK = K_LOC // 128           # 32 chunks (8 routes x 16 i each)
JO = J * O                      # 1024
F32 = mybir.dt.float32
BF16 = mybir.dt.bfloat16
FP16 = mybir.dt.float16
AX = mybir.AxisListType
ALU = mybir.AluOpType
ACTF = mybir.ActivationFunctionType

WC_ON_GPSIMD = lambda b: b < 5          # 4-chunk wc batches: 5 gps-direct, 3 ACT-expand+DVE
TREE_ON_GPSIMD = lambda b: False        # o-reduce trees stay on vector
MULT_ON_GPSIMD = lambda b: False        # all W.G multiplies on vector


def _build_nc():
    nc = bacc.Bacc(trn_type="TRN2", target_bir_lowering=False, debug=False,
                   num_devices=N_CORES)
    wt = nc.dram_tensor("wt", [K_LOC, JO], FP16, kind="ExternalInput")
    ut = nc.dram_tensor("ut", [K_LOC, BS], FP16, kind="ExternalInput")
    un = nc.dram_tensor("un", [BS, K_LOC], FP16, kind="ExternalInput")
    sel = nc.dram_tensor("sel", [128, 128], FP16, kind="ExternalInput")
    vout = nc.dram_tensor("vout", [BS, JO], F32, kind="ExternalOutput")
    cc_wi = nc.dram_tensor("cc_wi", [1, 128], F32)
    cc_wo = nc.dram_tensor("cc_wo", [1, 128], F32, addr_space="Shared")
    cc_in = [nc.dram_tensor(f"cc_in{i}", [BS, JO], FP16) for i in range(NUM_IT)]
    cc_out = [nc.dram_tensor(f"cc_out{i}", [BS, JO], FP16, addr_space="Shared")
              for i in range(NUM_IT)]
    rg = [list(range(N_CORES))]

    with tile.TileContext(nc) as tc:
        with (
            tc.tile_pool(name="big", bufs=1) as big,
            tc.tile_pool(name="wc", bufs=3) as wcp,
            tc.tile_pool(name="cx", bufs=2) as cxp,
            tc.tile_pool(name="tmp", bufs=2) as tmpp,
            tc.tile_pool(name="gsb", bufs=3) as gsbp,
            tc.tile_pool(name="small", bufs=1) as small,
            tc.tile_pool(name="ps", bufs=3, space="PSUM") as psp,
            tc.tile_pool(name="bpsum", bufs=1, space="PSUM") as bpsum,
        ):
            # ---- resident tensors ----
            w_sb = big.tile([128, NCHUNK, JO], FP16)      # 64KB/part
            ut_sb = big.tile([128, NCHUNK, BS], FP16)
            un_sb = big.tile([BS, K_LOC], FP16)
            sel_sb = big.tile([128, 128], FP16)            # selector (1/64)
            e_rep = big.tile([128, NCHUNK, J], F32)       # exp(b) scratch
            c_rep = big.tile([128, NCHUNK, J], F32)       # c_ij replicated
            b_acc = bpsum.tile([128, NCHUNK, J], F32)     # persistent b_ij

            wt_v = wt.ap().rearrange("(c p) f -> c p f", p=128)
            ut_v = ut.ap().rearrange("(c p) f -> c p f", p=128)
            _dengs = [nc.sync, nc.scalar, nc.gpsimd]
            for c in range(NCHUNK):
                _dengs[c % 3].dma_start(out=w_sb[:, c, :], in_=wt_v[c])
            for c in range(NCHUNK):
                _dengs[c % 3].dma_start(out=ut_sb[:, c, :], in_=ut_v[c])
            nc.sync.dma_start(out=un_sb, in_=un.ap())
            nc.sync.dma_start(out=sel_sb, in_=sel.ap())
            # warm up the collective machinery under the weight load
            nc.gpsimd.collective_compute(
                "AllReduce", ALU.add, replica_groups=rg,
                ins=[cc_wi.ap()], outs=[cc_wo.ap()],
            )

            def emit_pass1(it):
                """c-scale + s-matmul accumulation for iteration `it`."""
                s_full = psp.tile([128, JO], F32, tag="ps")
                s_ps = s_full[:BS, :]
                for b in range(NCHUNK // 4):
                    c0 = 4 * b
                    if it == 0:
                        rhs_src = w_sb[:, c0:c0 + 4, :]
                    elif WC_ON_GPSIMD(b):
                        wc_t = wcp.tile([128, 4, JO], FP16)
                        for hb in range(2):
                            cs = c0 + 2 * hb
                            nc.gpsimd.tensor_tensor(
                                out=wc_t[:, 2 * hb:2 * hb + 2, :].rearrange(
                                    "p c (j o) -> p c j o", o=O),
                                in0=w_sb[:, cs:cs + 2, :].rearrange(
                                    "p c (j o) -> p c j o", o=O),
                                in1=c_rep[:, cs:cs + 2, :].unsqueeze(3)
                                    .broadcast_to([128, 2, J, O]),
                                op=ALU.mult)
                        rhs_src = wc_t
                    else:
                        c_exp = cxp.tile([128, 4, JO], FP16)
                        nc.scalar.copy(
                            c_exp.rearrange("p c (j o) -> p c j o", o=O),
                            c_rep[:, c0:c0 + 4, :].unsqueeze(3)
                                .broadcast_to([128, 4, J, O]))
                        wc_t = wcp.tile([128, 4, JO], FP16)
                        nc.vector.tensor_tensor(
                            out=wc_t, in0=w_sb[:, c0:c0 + 4, :], in1=c_exp,
                            op=ALU.mult)
                        rhs_src = wc_t
                    for ci in range(4):
                        for h in range(2):
                            nc.tensor.matmul(
                                out=s_ps[:, h * 512:(h + 1) * 512],
                                lhsT=ut_sb[:, c0 + ci, :],
                                rhs=rhs_src[:, ci, h * 512:(h + 1) * 512],
                                start=(c0 + ci == 0),
                                stop=(c0 + ci == NCHUNK - 1))
                return s_ps

            def emit_ar_squash(it, s_ps):
                """psum->AR->squash; returns v_sb [BS, J, O] f32."""
                s_sb = small.tile([BS, JO], FP16, tag="s_sb")
                if it == 0:
                    nc.scalar.mul(s_sb, s_ps, 1.0 / J)
                else:
                    nc.scalar.copy(s_sb, s_ps)
                for q in range(4):
                    deng = nc.sync if q % 2 == 0 else nc.scalar
                    deng.dma_start(
                        out=cc_in[it].ap()[:, q * 256:(q + 1) * 256],
                        in_=s_sb[:, q * 256:(q + 1) * 256])
                nc.gpsimd.collective_compute(
                    "AllReduce", ALU.add, replica_groups=rg,
                    ins=[cc_in[it].ap()], outs=[cc_out[it].ap()])
                s2 = small.tile([BS, J, O], FP16, tag=f"s2_{it % 2}")
                s2f = s2.rearrange("p j o -> p (j o)")
                for q in range(4):
                    deng = nc.sync if q % 2 == 0 else nc.scalar
                    deng.dma_start(
                        out=s2f[:, q * 256:(q + 1) * 256],
                        in_=cc_out[it].ap()[:, q * 256:(q + 1) * 256])
                ss = small.tile([BS, J, O], F32, tag="s_sb")
                nc.scalar.square(ss, s2)
                sq = small.tile([BS, J], F32)
                nc.vector.tensor_reduce(out=sq, in_=ss, axis=AX.X, op=ALU.add)
                rt = small.tile([BS, J], F32)
                nc.scalar.activation(rt, sq, ACTF.Sqrt)
                op1 = small.tile([BS, J], F32)
                nc.vector.tensor_scalar_add(op1, sq, 1.0)
                rden = small.tile([BS, J], F32)
                nc.vector.reciprocal(rden, op1)
                fac = small.tile([BS, J], F32)
                nc.vector.tensor_tensor(out=fac, in0=rt, in1=rden,
                                        op=ALU.mult)
                v_sb = small.tile([BS, J, O], F32, tag=f"v_{it % 2}")
                nc.vector.tensor_tensor(
                    out=v_sb, in0=s2,
                    in1=fac.unsqueeze(2).broadcast_to([BS, J, O]),
                    op=ALU.mult)
                return v_sb

            def emit_pass2(it, v_sb):
                """b_ij update + per-batch softmax refresh of c_rep."""
                v_r = small.tile([BS, JO], FP16, tag=f"vr{it % 2}")
                nc.scalar.copy(v_r, v_sb.rearrange("p j o -> p (j o)"))
                NB = NCHUNK // 4
                for b in range(NB):
                    c0 = 4 * b
                    g_sb = gsbp.tile([128, 4, JO], FP16)
                    for ci in range(4):
                        g_ps = psp.tile([128, JO], F32, tag="ps")
                        for h in range(2):
                            nc.tensor.matmul(
                                out=g_ps[:, h * 512:(h + 1) * 512],
                                lhsT=un_sb[:, (c0 + ci) * 128:
                                           (c0 + ci + 1) * 128],
                                rhs=v_r[:, h * 512:(h + 1) * 512],
                                start=True, stop=True)
                        nc.scalar.copy(g_sb[:, ci, :], g_ps)
                    tmp = tmpp.tile([128, 4, JO], FP16)
                    meng = nc.gpsimd if MULT_ON_GPSIMD(b) else nc.vector
                    meng.tensor_tensor(out=tmp, in0=w_sb[:, c0:c0 + 4, :],
                                       in1=g_sb, op=ALU.mult)
                    # o-reduction: two pairwise stages then selector matmuls
                    teng = nc.gpsimd if TREE_ON_GPSIMD(b) else nc.vector
                    ta = tmpp.tile([128, 128, 16], FP16, tag="ta")
                    t0 = tmp.rearrange("p c (j o) -> p (c j) o", o=O)
                    teng.tensor_tensor(out=ta, in0=t0[:, :, 0:16],
                                       in1=t0[:, :, 16:32], op=ALU.add)
                    tb = tmpp.tile([128, 128, 8], FP16, tag="tb")
                    teng.tensor_tensor(out=tb, in0=ta[:, :, 0:8],
                                       in1=ta[:, :, 8:16], op=ALU.add)
                    tcq = tmpp.tile([128, 128, 4], FP16, tag="tc")
                    teng.tensor_tensor(out=tcq, in0=tb[:, :, 0:4],
                                       in1=tb[:, :, 4:8], op=ALU.add)
                    td = tmpp.tile([128, 128, 2], FP16, tag="td")
                    teng.tensor_tensor(out=td, in0=tcq[:, :, 0:2],
                                       in1=tcq[:, :, 2:4], op=ALU.add)
                    for oo in range(2):
                        nc.tensor.matmul(
                            out=b_acc[:, c0:c0 + 4, :], lhsT=sel_sb,
                            rhs=td[:, :, oo],
                            start=(it == 0 and c0 % 16 == 0 and oo == 0),
                            stop=(it == NUM_IT - 2 and c0 % 16 == 12
                                  and oo == 1),
                            skip_group_check=True)
                    # softmax refresh for this batch (4 chunks)
                    nc.scalar.activation(e_rep[:, c0:c0 + 4, :],
                                         b_acc[:, c0:c0 + 4, :], ACTF.Exp)
                    esum = tmpp.tile([128, 4], F32, tag="esum")
                    nc.vector.tensor_reduce(
                        out=esum, in_=e_rep[:, c0:c0 + 4, :],
                        axis=AX.X, op=ALU.add)
                    erec = tmpp.tile([128, 4], F32, tag="erec")
                    nc.vector.reciprocal(erec, esum)
                    for cc in range(4):
                        nc.scalar.mul(c_rep[:, c0 + cc, :],
                                      e_rep[:, c0 + cc, :],
                                      erec[:, cc:cc + 1])

            v_sb = None
            for it in range(NUM_IT):
                s_ps = emit_pass1(it)
                v_sb = emit_ar_squash(it, s_ps)
                if it < NUM_IT - 1:
                    emit_pass2(it, v_sb)

            v_flat_out = v_sb.rearrange("p j o -> p (j o)")
            for q in range(4):
                nc.sync.dma_start(out=vout.ap()[:, q * 256:(q + 1) * 256],
                                  in_=v_flat_out[:, q * 256:(q + 1) * 256])
    nc.finalize()
    return nc


_NC_CACHE = {}
TRACE = False
TRACE_CORES = None


def _get_nc():
    if "nc" not in _NC_CACHE:
        _NC_CACHE["nc"] = _build_nc()
    return _NC_CACHE["nc"]


def _make_sel():
    sel = np.zeros((128, 128), np.float32)
    for p in range(128):
        m0 = (p // 16) * 16
        sel[p, m0:m0 + 16] = 1.0 / BS
    return sel


def kernel(**inputs):
    in_caps = np.ascontiguousarray(inputs["in_caps"], dtype=np.float32)
    W = np.ascontiguousarray(inputs["W"], dtype=np.float32)
    assert in_caps.shape == (BS, R, I) and W.shape == (R, J, O, I)

    bf = np.float16
    Wt = np.ascontiguousarray(
        W.transpose(0, 3, 1, 2).reshape(R * I, J * O).astype(bf))
    uT = np.ascontiguousarray(
        in_caps.transpose(1, 2, 0).reshape(R * I, BS).astype(bf))
    un = np.ascontiguousarray(in_caps.reshape(BS, R * I).astype(bf))
    sel = _make_sel().astype(np.float16)

    in_maps = []
    for k in range(N_CORES):
        rows = slice(k * K_LOC, (k + 1) * K_LOC)
        in_maps.append({
            "wt": np.ascontiguousarray(Wt[rows]),
            "ut": np.ascontiguousarray(uT[rows]),
            "un": np.ascontiguousarray(un[:, rows]),
            "sel": sel,
        })

    nc = _get_nc()
    res = run_bass_kernel_spmd(nc, in_maps, core_ids=list(range(N_CORES)),
                               trace=TRACE, trace_cores=TRACE_CORES)
    _NC_CACHE["last_result"] = res
    v = np.asarray(res.results[0]["vout"], dtype=np.float32)
    return v.reshape(BS, J, O, 1)


if __name__ == "__main__":
    rng = np.random.default_rng(0)
    ins = {
        "in_caps": rng.standard_normal((BS, R, I), dtype=np.float32),
        "W": rng.standard_normal((R, J, O, I), dtype=np.float32),
    }
    out = kernel(**ins)
    print(out.shape, out.dtype, np.abs(out).mean())
